# revision 21
# baseline (speedup 1.0000x reference)
"""AFT-Full (Attention Free Transformer) Trainium2 kernel, 8 NeuronCores.

Reference computation (per batch b):
    qp  = q  @ qW_w.T + qW_b                    # [T, H]
    k   = kv @ kW_w.T + kW_b                    # [J, H]
    v   = kv @ vW_w.T + vW_b                    # [J, H]
    num = exp(w_bias) @ (exp(k) * v)            # [T, H]
    den = exp(w_bias) @ exp(k)                  # [T, H]
    out = sigmoid(qp) * num / den

Two exact algebraic simplifications used here:
  * exp(kW_b) factors out of both num and den (independent of j) and
    cancels in num/den -> kW_b is dropped entirely.
  * the vW_b contribution to num is vW_b[h] * den, so
    num/den = num_nobias/den + vW_b -> vW_b becomes an epilogue add.

Sharding: 8 cores = 4 batch-groups (2 batches each) x 2 t-halves (2048
rows of w_bias each). No collectives; each core reads exactly what it
needs. All device matmuls run in bf16 (fp32 PSUM accumulation).

Host-side prep only slices/permutes inputs so that (a) contraction dims
land on the SBUF partition axis and (b) every DMA reads a fully
contiguous DRAM block (strided 2KB-line reads measured ~40% slower).
"""

import sys

import numpy as np

if "/opt/trn_rl_repo" not in sys.path:  # harness env has it via
    sys.path.append("/opt/trn_rl_repo")    # sitecustomize; belt+braces

# Full-problem geometry (hardcoded per harness contract).
B, T, J, C, H = 8, 4096, 4096, 512, 128
N_CORES = 8
BG = 4          # batch groups (cores c and c+4 share batches [2c, 2c+1])
B2 = B // BG    # batches per core
TC = T // 2     # t rows per core (2048)
TBLK = 512      # t block (psum free dim)
JBLK = 512      # j chunk for kv load / proj
P = 128
JQ = 4          # j-tiles per w_bias DMA (quad)
EWPAIR = 2      # j-tiles per ew bf16 tile / ACT op


def _prep(q_s, kv_s, wb_s, tblk, jblk):
    """Rearrange raw per-core shards into contiguous-DMA-block layouts.

    q_s  [b2, tc, C] -> [b2, nt, P, ncc*tblk]   partition-major blocks
    kv_s [b2, J,  C] -> [b2, njc, P, ncc*jblk]  (8KB contiguous per line)
    wb_s [tc, J]     -> [nt, njq, P, JQ*tblk]
    """
    b2, tc_, c = q_s.shape
    j = kv_s.shape[1]
    nt, njc, njq = tc_ // tblk, j // jblk, j // (JQ * P)
    ncc = c // P
    qr = np.ascontiguousarray(
        q_s.reshape(b2, nt, tblk, ncc, P).transpose(0, 1, 4, 3, 2),
        dtype=np.float32).reshape(b2, nt, P, ncc * tblk)
    kvr = np.ascontiguousarray(
        kv_s.reshape(b2, njc, jblk, ncc, P).transpose(0, 1, 4, 3, 2),
        dtype=np.float32).reshape(b2, njc, P, ncc * jblk)
    wbr = np.ascontiguousarray(
        wb_s.reshape(nt, tblk, njq, JQ, P).transpose(0, 2, 4, 3, 1),
        dtype=np.float32).reshape(nt, njq, P, JQ * tblk)
    return qr, kvr, wbr


def _build_graph(b2, tc_, j, c, h, tblk, jblk):
    import concourse.bass as bass  # noqa: F401
    import concourse.mybir as mybir
    import concourse.tile as tile
    from concourse import bacc

    f32 = mybir.dt.float32
    bf16 = mybir.dt.bfloat16
    AF = mybir.ActivationFunctionType

    nt = tc_ // tblk          # t blocks per core
    njc = j // jblk           # j chunks (kv/proj granularity)
    njt = j // P              # j tiles (matmul granularity)
    njq = njt // JQ           # w_bias DMA quads per t block
    jt_per_chunk = jblk // P
    ncc = c // P              # contraction tiles for projections

    nc = bacc.Bacc("TRN2", target_bir_lowering=False, debug=False,
                   num_devices=N_CORES)

    qT_e = nc.declare_dram_parameter("qT", [b2, nt, P, ncc * tblk], f32,
                                     isOutput=False)
    kvT_e = nc.declare_dram_parameter("kvT", [b2, njc, P, ncc * jblk], f32,
                                      isOutput=False)
    wbT_e = nc.declare_dram_parameter("wbT", [nt, njq, P, JQ * tblk], f32,
                                      isOutput=False)
    wkv_e = nc.declare_dram_parameter("wkv", [c, 2 * h], f32, isOutput=False)
    qWT_e = nc.declare_dram_parameter("qWT", [c, h], f32, isOutput=False)
    qb_e = nc.declare_dram_parameter("qb", [h, 1], f32, isOutput=False)
    vb_e = nc.declare_dram_parameter("vb", [h, 1], f32, isOutput=False)
    out_e = nc.declare_dram_parameter("out", [b2, nt, h, tblk], f32,
                                      isOutput=True)

    with tile.TileContext(nc) as tc:
        with (
            tc.tile_pool(name="wstage", bufs=2) as wstage,
            tc.tile_pool(name="wbf", bufs=ncc) as wbf,
            tc.tile_pool(name="bias", bufs=1) as biasp,
            tc.tile_pool(name="kstage", bufs=3) as kstage,
            tc.tile_pool(name="kbf", bufs=5) as kbf,
            tc.tile_pool(name="slab", bufs=b2) as slabp,
            tc.tile_pool(name="sig", bufs=b2) as sigp,
            tc.tile_pool(name="ewstage", bufs=3) as ewstage,
            tc.tile_pool(name="ewbf", bufs=njq * (JQ // EWPAIR) + 4
                         ) as ewbf,
            tc.tile_pool(name="ep", bufs=4) as epp,
            tc.tile_pool(name="vt", bufs=6) as vtp,
            tc.tile_pool(name="outst", bufs=4) as outst,
            tc.tile_pool(name="pp", bufs=2, space="PSUM") as projps,
            tc.tile_pool(name="s2", bufs=6, space="PSUM") as s2ps,
        ):
            npair = JQ // EWPAIR
            # ---- head-start: first kv chunk DMA before anything ----
            ks00 = kstage.tile([P, ncc * jblk], f32, tag="kst", name="ks00")
            nc.sync.dma_start(ks00[:], kvT_e[0, 0])

            # ---- weights ----
            wkv_bf = []
            qwt_bf = []
            for ct in range(ncc):
                wf = wstage.tile([P, 2 * h], f32, tag="wst", name=f"wf{ct}")
                nc.sync.dma_start(wf[:], wkv_e[ct * P:(ct + 1) * P, :])
                wb = wbf.tile([P, 2 * h], bf16, tag="wkv", name=f"wkvbf{ct}")
                nc.vector.tensor_copy(wb[:], wf[:])
                wkv_bf.append(wb)
            for ct in range(ncc):
                qf = wstage.tile([P, h], f32, tag="wst", name=f"qf{ct}")
                nc.sync.dma_start(qf[:], qWT_e[ct * P:(ct + 1) * P, :])
                qb_ = wbf.tile([P, h], bf16, tag="qwt", name=f"qwtbf{ct}")
                nc.vector.tensor_copy(qb_[:], qf[:])
                qwt_bf.append(qb_)
            qb_sb = biasp.tile([h, 1], f32, tag="qb")
            vb_sb = biasp.tile([h, 1], f32, tag="vb")
            nc.sync.dma_start(qb_sb[:], qb_e[:])
            nc.sync.dma_start(vb_sb[:], vb_e[:])
            qbh_sb = biasp.tile([h, 1], f32, tag="qbh")
            nc.scalar.mul(qbh_sb[:], qb_sb[:], 0.5)

            ek_slab, ekv_slab, sig_slab = [], [], []
            for b in range(b2):
                ek_slab.append(slabp.tile([P, j], bf16, tag="ek",
                                          name=f"ek{b}"))
                ekv_slab.append(slabp.tile([P, j], bf16, tag="ekv",
                                           name=f"ekv{b}"))
                sig_slab.append(sigp.tile([P, tc_], f32, tag="sig",
                                          name=f"sig{b}"))

            # ---- software-pipelined emission helpers ----
            ew_pairs = {}
            ew_seq = [(tb, jq) for tb in range(nt) for jq in range(njq)]
            ew_state = {"emitted": 0}

            def emit_ew_quad():
                idx = ew_state["emitted"]
                if idx >= len(ew_seq):
                    return
                ew_state["emitted"] = idx + 1
                tb, jq = ew_seq[idx]
                ews = ewstage.tile([P, JQ * tblk], f32, tag="ews",
                                   name=f"ews{tb}_{jq}")
                nc.sync.dma_start(ews[:], wbT_e[tb, jq])
                for ip in range(npair):
                    et = ewbf.tile([P, EWPAIR * tblk], bf16, tag="ewb",
                                   name=f"ewb{tb}_{jq}_{ip}")
                    nc.scalar.activation(
                        et[:],
                        ews[:, ip * EWPAIR * tblk:(ip + 1) * EWPAIR * tblk],
                        AF.Exp)
                    ew_pairs[(tb, jq * npair + ip)] = et

            def ensure_ew(tb):
                while ew_state["emitted"] < (tb + 1) * njq:
                    emit_ew_quad()

            def dribble_ew(tb_cap):
                if ew_state["emitted"] < tb_cap * njq:
                    emit_ew_quad()

            def emit_kv_chunk(b, jc):
                if b == 0 and jc == 0:
                    ks = ks00
                else:
                    ks = kstage.tile([P, ncc * jblk], f32, tag="kst",
                                     name=f"ks{b}_{jc}")
                    nc.sync.dma_start(ks[:], kvT_e[b, jc])
                kb = kbf.tile([P, ncc * jblk], bf16, tag="kbf",
                              name=f"kb{b}_{jc}")
                nc.vector.tensor_copy(kb[:], ks[:])
                for jt in range(jt_per_chunk):
                    ps = projps.tile([P, 2 * h], mybir.dt.float32, tag="pp",
                                     name=f"ps{b}_{jc}_{jt}")
                    for ct in range(ncc):
                        nc.tensor.matmul(
                            ps[:],
                            kb[:, ct * jblk + jt * P:ct * jblk + (jt + 1) * P],
                            wkv_bf[ct][:],
                            start=(ct == 0), stop=(ct == ncc - 1))
                    jg = jc * jblk + jt * P
                    nc.scalar.activation(
                        ek_slab[b][:, jg:jg + P], ps[:, 0:h], AF.Exp)
                    vtmp = vtp.tile([P, h], bf16, tag="vtmp")
                    nc.vector.tensor_copy(vtmp[:], ps[:, h:2 * h])
                    nc.vector.tensor_mul(
                        ekv_slab[b][:, jg:jg + P],
                        ek_slab[b][:, jg:jg + P], vtmp[:])

            def emit_qp(b, tb):
                # sigmoid(x+qb) = 0.5 + 0.5*tanh((x+qb)/2); tanh shares the
                # exp ACT table set, so no ~2.7us table switches.
                qs = kstage.tile([P, ncc * tblk], f32, tag="kst",
                                 name=f"qs{b}_{tb}")
                nc.sync.dma_start(qs[:], qT_e[b, tb])
                qbf_ = kbf.tile([P, ncc * tblk], bf16, tag="kbf",
                                name=f"qbf{b}_{tb}")
                nc.vector.tensor_copy(qbf_[:], qs[:])
                qps = projps.tile([P, tblk], mybir.dt.float32, tag="pp",
                                  name=f"qps{b}_{tb}")
                for ct in range(ncc):
                    nc.tensor.matmul(
                        qps[:], qwt_bf[ct][:],
                        qbf_[:, ct * tblk:(ct + 1) * tblk],
                        start=(ct == 0), stop=(ct == ncc - 1))
                sl = sig_slab[b][:, tb * tblk:(tb + 1) * tblk]
                nc.scalar.activation(sl, qps[:], AF.Tanh,
                                     bias=qbh_sb[:], scale=0.5)
                nc.vector.tensor_scalar(
                    sl, sl, 0.5, 0.5,
                    op0=mybir.AluOpType.mult, op1=mybir.AluOpType.add)

            def emit_pass_seg(tb, b, accn, accd, jc):
                for jt in range(jc * jt_per_chunk, (jc + 1) * jt_per_chunk):
                    rhs = ew_pairs[(tb, jt // EWPAIR)][
                        :, (jt % EWPAIR) * tblk:(jt % EWPAIR + 1) * tblk]
                    nc.tensor.matmul(
                        accn[:], ekv_slab[b][:, jt * P:(jt + 1) * P],
                        rhs, start=(jt == 0), stop=(jt == njt - 1))
                    nc.tensor.matmul(
                        accd[:], ek_slab[b][:, jt * P:(jt + 1) * P],
                        rhs, start=(jt == 0), stop=(jt == njt - 1))

            def emit_epi(tb, b, accn, accd):
                rden = epp.tile([P, tblk], f32, tag="rden")
                nc.vector.reciprocal(rden[:], accd[:])
                ratio = epp.tile([P, tblk], f32, tag="ratio")
                nc.vector.tensor_mul(ratio[:], accn[:], rden[:])
                ot = outst.tile([P, tblk], f32, tag="out")
                nc.vector.scalar_tensor_tensor(
                    ot[:], ratio[:], vb_sb[:],
                    sig_slab[b][:, tb * tblk:(tb + 1) * tblk],
                    op0=mybir.AluOpType.add, op1=mybir.AluOpType.mult)
                nc.scalar.dma_start(out_e[b, tb], ot[:])

            # ---- stage 1: kv projections (dribble tb0 ew quads) ----
            for b in range(b2):
                for jc in range(njc):
                    emit_kv_chunk(b, jc)
                    dribble_ew(1)

            # ---- stage 2: per t-block: qp proj, passes, epilogue ----
            for tb in range(nt):
                ensure_ew(tb)
                for b in range(b2):
                    emit_qp(b, tb)
                for b in range(b2):
                    accn = s2ps.tile([P, tblk], mybir.dt.float32, tag="s2",
                                     name=f"accn{tb}_{b}")
                    accd = s2ps.tile([P, tblk], mybir.dt.float32, tag="s2",
                                     name=f"accd{tb}_{b}")
                    for jc in range(njc):
                        emit_pass_seg(tb, b, accn, accd, jc)
                        dribble_ew(tb + 2)
                    emit_epi(tb, b, accn, accd)

    nc.compile()
    return nc


_NC_CACHE = {}


def _get_nc(key, args):
    if key not in _NC_CACHE:
        _NC_CACHE[key] = _build_graph(*args)
    return _NC_CACHE[key]


def kernel(q, kv, qW_w, qW_b, kW_w, kW_b, vW_w, vW_b, w_bias):
    from concourse.bass_utils import run_bass_kernel_spmd

    q = np.asarray(q, dtype=np.float32)
    kv = np.asarray(kv, dtype=np.float32)
    w_bias = np.asarray(w_bias, dtype=np.float32)

    wkv = np.ascontiguousarray(
        np.concatenate([np.asarray(kW_w).T, np.asarray(vW_w).T], axis=1),
        dtype=np.float32)                                   # [C, 2H]
    qWT = np.ascontiguousarray(np.asarray(qW_w).T, dtype=np.float32)  # [C, H]
    qb = np.asarray(qW_b, dtype=np.float32).reshape(H, 1).copy()
    vb = np.asarray(vW_b, dtype=np.float32).reshape(H, 1).copy()

    # kvT shared between the two cores of each batch group
    kv_groups = {}
    in_maps = []
    for core in range(N_CORES):
        g, th = core % BG, core // BG
        t0 = th * TC
        q_s = q[2 * g:2 * g + 2, t0:t0 + TC, :]
        kv_s = kv[2 * g:2 * g + 2]
        wb_s = w_bias[t0:t0 + TC, :]
        qr, kvr, wbr = _prep(q_s, kv_s, wb_s, TBLK, JBLK)
        if g in kv_groups:
            kvr = kv_groups[g]
        else:
            kv_groups[g] = kvr
        in_maps.append({
            "qT": qr, "kvT": kvr, "wbT": wbr,
            "wkv": wkv, "qWT": qWT, "qb": qb, "vb": vb,
        })

    nc = _get_nc("full", (B2, TC, J, C, H, TBLK, JBLK))
    res = run_bass_kernel_spmd(nc, in_maps, core_ids=list(range(N_CORES)))

    out = np.empty((B, T, H), dtype=np.float32)
    nt = TC // TBLK
    for core in range(N_CORES):
        g, th = core % BG, core // BG
        t0 = th * TC
        o = res.results[core]["out"]          # [2, nt, H, TBLK]
        for b in range(B2):
            for tb in range(nt):
                out[2 * g + b,
                    t0 + tb * TBLK:t0 + (tb + 1) * TBLK, :] = o[b, tb].T
    return out


# revision 22
# speedup vs baseline: 1.0188x; 1.0188x over previous
"""AFT-Full (Attention Free Transformer) Trainium2 kernel, 8 NeuronCores.

Reference computation (per batch b):
    qp  = q  @ qW_w.T + qW_b                    # [T, H]
    k   = kv @ kW_w.T + kW_b                    # [J, H]
    v   = kv @ vW_w.T + vW_b                    # [J, H]
    num = exp(w_bias) @ (exp(k) * v)            # [T, H]
    den = exp(w_bias) @ exp(k)                  # [T, H]
    out = sigmoid(qp) * num / den

Two exact algebraic simplifications used here:
  * exp(kW_b) factors out of both num and den (independent of j) and
    cancels in num/den -> kW_b is dropped entirely.
  * the vW_b contribution to num is vW_b[h] * den, so
    num/den = num_nobias/den + vW_b -> vW_b becomes an epilogue add.

Sharding: 8 cores = 4 batch-groups (2 batches each) x 2 t-halves (2048
rows of w_bias each). No collectives; each core reads exactly what it
needs. All device matmuls run in bf16 (fp32 PSUM accumulation).

Host-side prep only slices/permutes inputs so that (a) contraction dims
land on the SBUF partition axis and (b) every DMA reads a fully
contiguous DRAM block (strided 2KB-line reads measured ~40% slower).
"""

import sys

import numpy as np

if "/opt/trn_rl_repo" not in sys.path:  # harness env has it via
    sys.path.append("/opt/trn_rl_repo")    # sitecustomize; belt+braces

# Full-problem geometry (hardcoded per harness contract).
B, T, J, C, H = 8, 4096, 4096, 512, 128
N_CORES = 8
BG = 4          # batch groups (cores c and c+4 share batches [2c, 2c+1])
B2 = B // BG    # batches per core
TC = T // 2     # t rows per core (2048)
TBLK = 512      # t block (psum free dim)
JBLK = 512      # j chunk for kv load / proj
P = 128
JQ = 4          # j-tiles per w_bias DMA (quad)
EWPAIR = 2      # j-tiles per ew bf16 tile / ACT op


def _prep(q_s, kv_s, wb_s, tblk, jblk):
    """Rearrange raw per-core shards into contiguous-DMA-block layouts.

    q_s  [b2, tc, C] -> [b2, nt, P, ncc*tblk]   partition-major blocks
    kv_s [b2, J,  C] -> [b2, njc, P, ncc*jblk]  (8KB contiguous per line)
    wb_s [tc, J]     -> [nt, njq, P, JQ*tblk]
    """
    b2, tc_, c = q_s.shape
    j = kv_s.shape[1]
    nt, njc, njq = tc_ // tblk, j // jblk, j // (JQ * P)
    ncc = c // P
    qr = np.ascontiguousarray(
        q_s.reshape(b2, nt, tblk, ncc, P).transpose(0, 1, 4, 3, 2),
        dtype=np.float32).reshape(b2, nt, P, ncc * tblk)
    kvr = np.ascontiguousarray(
        kv_s.reshape(b2, njc, jblk, ncc, P).transpose(0, 1, 4, 3, 2),
        dtype=np.float32).reshape(b2, njc, P, ncc * jblk)
    wbr = np.ascontiguousarray(
        wb_s.reshape(nt, tblk, njq, JQ, P).transpose(0, 2, 4, 3, 1),
        dtype=np.float32).reshape(nt, njq, P, JQ * tblk)
    return qr, kvr, wbr


def _build_graph(b2, tc_, j, c, h, tblk, jblk):
    import concourse.bass as bass  # noqa: F401
    import concourse.mybir as mybir
    import concourse.tile as tile
    from concourse import bacc

    f32 = mybir.dt.float32
    bf16 = mybir.dt.bfloat16
    AF = mybir.ActivationFunctionType

    nt = tc_ // tblk          # t blocks per core
    njc = j // jblk           # j chunks (kv/proj granularity)
    njt = j // P              # j tiles (matmul granularity)
    njq = njt // JQ           # w_bias DMA quads per t block
    jt_per_chunk = jblk // P
    ncc = c // P              # contraction tiles for projections

    nc = bacc.Bacc("TRN2", target_bir_lowering=False, debug=False,
                   num_devices=N_CORES)

    qT_e = nc.declare_dram_parameter("qT", [b2, nt, P, ncc * tblk], f32,
                                     isOutput=False)
    kvT_e = nc.declare_dram_parameter("kvT", [b2, njc, P, ncc * jblk], f32,
                                      isOutput=False)
    wbT_e = nc.declare_dram_parameter("wbT", [nt, njq, P, JQ * tblk], f32,
                                      isOutput=False)
    wkv_e = nc.declare_dram_parameter("wkv", [c, 2 * h], f32, isOutput=False)
    qWT_e = nc.declare_dram_parameter("qWT", [c, h], f32, isOutput=False)
    qb_e = nc.declare_dram_parameter("qb", [h, 1], f32, isOutput=False)
    vb_e = nc.declare_dram_parameter("vb", [h, 1], f32, isOutput=False)
    out_e = nc.declare_dram_parameter("out", [b2, nt, h, tblk], f32,
                                      isOutput=True)

    with tile.TileContext(nc) as tc:
        with (
            tc.tile_pool(name="wstage", bufs=2) as wstage,
            tc.tile_pool(name="wbf", bufs=ncc) as wbf,
            tc.tile_pool(name="bias", bufs=1) as biasp,
            tc.tile_pool(name="kstage", bufs=3) as kstage,
            tc.tile_pool(name="kbf", bufs=4) as kbf,
            tc.tile_pool(name="slab", bufs=b2) as slabp,
            tc.tile_pool(name="sig", bufs=b2) as sigp,
            tc.tile_pool(name="ewstage", bufs=3) as ewstage,
            tc.tile_pool(name="ewbf", bufs=njq * (JQ // EWPAIR) + 4
                         ) as ewbf,
            tc.tile_pool(name="ep", bufs=4) as epp,
            tc.tile_pool(name="vt", bufs=6) as vtp,
            tc.tile_pool(name="outst", bufs=4) as outst,
            tc.tile_pool(name="pp", bufs=2, space="PSUM") as projps,
            tc.tile_pool(name="s2", bufs=6, space="PSUM") as s2ps,
        ):
            npair = JQ // EWPAIR
            # ---- head-start: first kv chunk DMA before anything ----
            ks00 = kstage.tile([P, ncc * jblk], f32, tag="kst", name="ks00")
            nc.sync.dma_start(ks00[:], kvT_e[0, 0])

            # ---- weights ----
            wkv_bf = []
            qwt_bf = []
            for ct in range(ncc):
                wf = wstage.tile([P, 2 * h], f32, tag="wst", name=f"wf{ct}")
                nc.sync.dma_start(wf[:], wkv_e[ct * P:(ct + 1) * P, :])
                wb = wbf.tile([P, 2 * h], bf16, tag="wkv", name=f"wkvbf{ct}")
                nc.vector.tensor_copy(wb[:], wf[:])
                wkv_bf.append(wb)
            for ct in range(ncc):
                qf = wstage.tile([P, h], f32, tag="wst", name=f"qf{ct}")
                nc.sync.dma_start(qf[:], qWT_e[ct * P:(ct + 1) * P, :])
                qb_ = wbf.tile([P, h], bf16, tag="qwt", name=f"qwtbf{ct}")
                nc.vector.tensor_copy(qb_[:], qf[:])
                qwt_bf.append(qb_)
            qb_sb = biasp.tile([h, 1], f32, tag="qb")
            vb_sb = biasp.tile([h, 1], f32, tag="vb")
            nc.sync.dma_start(qb_sb[:], qb_e[:])
            nc.sync.dma_start(vb_sb[:], vb_e[:])
            qbh_sb = biasp.tile([h, 1], f32, tag="qbh")
            nc.scalar.mul(qbh_sb[:], qb_sb[:], 0.5)

            ek_slab, ekv_slab, sig_slab = [], [], []
            for b in range(b2):
                ek_slab.append(slabp.tile([P, j], bf16, tag="ek",
                                          name=f"ek{b}"))
                ekv_slab.append(slabp.tile([P, j], bf16, tag="ekv",
                                           name=f"ekv{b}"))
                sig_slab.append(sigp.tile([P, tc_], f32, tag="sig",
                                          name=f"sig{b}"))

            # ---- software-pipelined emission helpers ----
            ew_pairs = {}
            ew_seq = [(tb, jq) for tb in range(nt) for jq in range(njq)]
            ew_state = {"emitted": 0}

            def emit_ew_quad():
                idx = ew_state["emitted"]
                if idx >= len(ew_seq):
                    return
                ew_state["emitted"] = idx + 1
                tb, jq = ew_seq[idx]
                ews = ewstage.tile([P, JQ * tblk], f32, tag="ews",
                                   name=f"ews{tb}_{jq}")
                nc.sync.dma_start(ews[:], wbT_e[tb, jq])
                for ip in range(npair):
                    et = ewbf.tile([P, EWPAIR * tblk], bf16, tag="ewb",
                                   name=f"ewb{tb}_{jq}_{ip}")
                    nc.scalar.activation(
                        et[:],
                        ews[:, ip * EWPAIR * tblk:(ip + 1) * EWPAIR * tblk],
                        AF.Exp)
                    ew_pairs[(tb, jq * npair + ip)] = et

            def ensure_ew(tb):
                while ew_state["emitted"] < (tb + 1) * njq:
                    emit_ew_quad()

            def dribble_ew(tb_cap):
                if ew_state["emitted"] < tb_cap * njq:
                    emit_ew_quad()

            def emit_kv_chunk(b, jc):
                if b == 0 and jc == 0:
                    ks = ks00
                else:
                    ks = kstage.tile([P, ncc * jblk], f32, tag="kst",
                                     name=f"ks{b}_{jc}")
                    nc.sync.dma_start(ks[:], kvT_e[b, jc])
                kb = kbf.tile([P, ncc * jblk], bf16, tag="kbf",
                              name=f"kb{b}_{jc}")
                nc.vector.tensor_copy(kb[:], ks[:])
                for jt in range(jt_per_chunk):
                    ps = projps.tile([P, 2 * h], mybir.dt.float32, tag="pp",
                                     name=f"ps{b}_{jc}_{jt}")
                    for ct in range(ncc):
                        nc.tensor.matmul(
                            ps[:],
                            kb[:, ct * jblk + jt * P:ct * jblk + (jt + 1) * P],
                            wkv_bf[ct][:],
                            start=(ct == 0), stop=(ct == ncc - 1))
                    jg = jc * jblk + jt * P
                    nc.scalar.activation(
                        ek_slab[b][:, jg:jg + P], ps[:, 0:h], AF.Exp)
                    vtmp = vtp.tile([P, h], bf16, tag="vtmp")
                    nc.vector.tensor_copy(vtmp[:], ps[:, h:2 * h])
                    nc.vector.tensor_mul(
                        ekv_slab[b][:, jg:jg + P],
                        ek_slab[b][:, jg:jg + P], vtmp[:])

            def emit_qp(b, tb):
                # sigmoid(x+qb) = 0.5 + 0.5*tanh((x+qb)/2); tanh shares the
                # exp ACT table set, so no ~2.7us table switches.
                qs = kstage.tile([P, ncc * tblk], f32, tag="kst",
                                 name=f"qs{b}_{tb}")
                nc.sync.dma_start(qs[:], qT_e[b, tb])
                qbf_ = kbf.tile([P, ncc * tblk], bf16, tag="kbf",
                                name=f"qbf{b}_{tb}")
                nc.vector.tensor_copy(qbf_[:], qs[:])
                qps = projps.tile([P, tblk], mybir.dt.float32, tag="pp",
                                  name=f"qps{b}_{tb}")
                for ct in range(ncc):
                    nc.tensor.matmul(
                        qps[:], qwt_bf[ct][:],
                        qbf_[:, ct * tblk:(ct + 1) * tblk],
                        start=(ct == 0), stop=(ct == ncc - 1))
                sl = sig_slab[b][:, tb * tblk:(tb + 1) * tblk]
                nc.scalar.activation(sl, qps[:], AF.Tanh,
                                     bias=qbh_sb[:], scale=0.5)
                nc.vector.tensor_scalar(
                    sl, sl, 0.5, 0.5,
                    op0=mybir.AluOpType.mult, op1=mybir.AluOpType.add)

            def emit_pass_seg(tb, b, accn, accd, jc):
                for jt in range(jc * jt_per_chunk, (jc + 1) * jt_per_chunk):
                    rhs = ew_pairs[(tb, jt // EWPAIR)][
                        :, (jt % EWPAIR) * tblk:(jt % EWPAIR + 1) * tblk]
                    nc.tensor.matmul(
                        accn[:], ekv_slab[b][:, jt * P:(jt + 1) * P],
                        rhs, start=(jt == 0), stop=(jt == njt - 1))
                    nc.tensor.matmul(
                        accd[:], ek_slab[b][:, jt * P:(jt + 1) * P],
                        rhs, start=(jt == 0), stop=(jt == njt - 1))

            def emit_epi(tb, b, accn, accd):
                rden = epp.tile([P, tblk], f32, tag="rden")
                nc.vector.reciprocal(rden[:], accd[:])
                ratio = epp.tile([P, tblk], f32, tag="ratio")
                nc.vector.tensor_mul(ratio[:], accn[:], rden[:])
                ot = outst.tile([P, tblk], f32, tag="out")
                nc.vector.scalar_tensor_tensor(
                    ot[:], ratio[:], vb_sb[:],
                    sig_slab[b][:, tb * tblk:(tb + 1) * tblk],
                    op0=mybir.AluOpType.add, op1=mybir.AluOpType.mult)
                nc.scalar.dma_start(out_e[b, tb], ot[:])

            # ---- stage 1: kv projections (dribble tb0 ew quads) ----
            for b in range(b2):
                for jc in range(njc):
                    emit_kv_chunk(b, jc)
                    dribble_ew(1)

            # ---- stage 2: per t-block: qp proj, passes, epilogue ----
            for tb in range(nt):
                ensure_ew(tb)
                for b in range(b2):
                    emit_qp(b, tb)
                for b in range(b2):
                    accn = s2ps.tile([P, tblk], mybir.dt.float32, tag="s2",
                                     name=f"accn{tb}_{b}")
                    accd = s2ps.tile([P, tblk], mybir.dt.float32, tag="s2",
                                     name=f"accd{tb}_{b}")
                    for jc in range(njc):
                        emit_pass_seg(tb, b, accn, accd, jc)
                        dribble_ew(tb + 2)
                    emit_epi(tb, b, accn, accd)

    nc.compile()
    return nc


_NC_CACHE = {}


def _get_nc(key, args):
    if key not in _NC_CACHE:
        _NC_CACHE[key] = _build_graph(*args)
    return _NC_CACHE[key]


def kernel(q, kv, qW_w, qW_b, kW_w, kW_b, vW_w, vW_b, w_bias):
    from concourse.bass_utils import run_bass_kernel_spmd

    q = np.asarray(q, dtype=np.float32)
    kv = np.asarray(kv, dtype=np.float32)
    w_bias = np.asarray(w_bias, dtype=np.float32)

    wkv = np.ascontiguousarray(
        np.concatenate([np.asarray(kW_w).T, np.asarray(vW_w).T], axis=1),
        dtype=np.float32)                                   # [C, 2H]
    qWT = np.ascontiguousarray(np.asarray(qW_w).T, dtype=np.float32)  # [C, H]
    qb = np.asarray(qW_b, dtype=np.float32).reshape(H, 1).copy()
    vb = np.asarray(vW_b, dtype=np.float32).reshape(H, 1).copy()

    # kvT shared between the two cores of each batch group
    kv_groups = {}
    in_maps = []
    for core in range(N_CORES):
        g, th = core % BG, core // BG
        t0 = th * TC
        q_s = q[2 * g:2 * g + 2, t0:t0 + TC, :]
        kv_s = kv[2 * g:2 * g + 2]
        wb_s = w_bias[t0:t0 + TC, :]
        qr, kvr, wbr = _prep(q_s, kv_s, wb_s, TBLK, JBLK)
        if g in kv_groups:
            kvr = kv_groups[g]
        else:
            kv_groups[g] = kvr
        in_maps.append({
            "qT": qr, "kvT": kvr, "wbT": wbr,
            "wkv": wkv, "qWT": qWT, "qb": qb, "vb": vb,
        })

    nc = _get_nc("full", (B2, TC, J, C, H, TBLK, JBLK))
    res = run_bass_kernel_spmd(nc, in_maps, core_ids=list(range(N_CORES)))

    out = np.empty((B, T, H), dtype=np.float32)
    nt = TC // TBLK
    for core in range(N_CORES):
        g, th = core % BG, core // BG
        t0 = th * TC
        o = res.results[core]["out"]          # [2, nt, H, TBLK]
        for b in range(B2):
            for tb in range(nt):
                out[2 * g + b,
                    t0 + tb * TBLK:t0 + (tb + 1) * TBLK, :] = o[b, tb].T
    return out
